# revision 1
# baseline (speedup 1.0000x reference)
"""Trainium2 Bass kernel for nn_MultiHeadAttention_56375740727430.

Causal multi-head attention, B=2 S=2048 D=1024 H=16 KS=64, followed by an
output projection `heads @ kernel`.

Sharding: pure data/head parallel over 8 cores — core c handles batch c//4
and 4 heads (c%4)*4 ... +4.  Each core computes Q^T/K^T (head-pair-stacked,
transposed layout), V (natural layout, with an appended ones-column so the
softmax denominator Z falls out of the attention matmul), causal scores ->
exp -> (P@V | Z) -> per-head output projection, all unnormalized.  The host
divides by Z, sums head contributions and batch-partials, and transposes.

Matmul operands are bf16 (1 cycle/row on the PE); accumulation, scores (exp
input) and Z stay fp32.
"""

import sys

sys.path.insert(0, "/opt/trn_rl_repo")

from contextlib import ExitStack

import ml_dtypes
import numpy as np

import concourse.bass as bass
import concourse.bacc as bacc
import concourse.mybir as mybir
import concourse.tile as tile

B, S, D = 2, 2048, 1024
H, KS = 16, 64

P = 128            # partitions
NCORES = 8
CORES_PER_B = NCORES // B          # 4
NH = H // CORES_PER_B              # heads per core = 4
NW = NH * KS                       # per-core projection width = 256
DT = D // P                        # d-tiles = 8
ST = S // P                        # s/l-tiles = 16
IB = 512                           # query block
NIB = S // IB                      # 4
LPB = IB // P                      # l-tiles per query block = 4

F32 = mybir.dt.float32
BF16 = mybir.dt.bfloat16
NP_BF16 = ml_dtypes.bfloat16
EXP = mybir.ActivationFunctionType.Exp


def build_nc():
    mm_dt = BF16
    nc = bacc.Bacc()

    xT = nc.declare_dram_parameter("xT", [D, S], mm_dt, isOutput=False)
    wq = nc.declare_dram_parameter("wq", [D, NW], mm_dt, isOutput=False)
    wk = nc.declare_dram_parameter("wk", [D, NW], mm_dt, isOutput=False)
    wv = nc.declare_dram_parameter("wv", [D, NW], mm_dt, isOutput=False)
    wkern = nc.declare_dram_parameter("wkern", [NH, KS, KS], mm_dt, isOutput=False)
    masks = nc.declare_dram_parameter("masks", [P, P], mm_dt, isOutput=False)
    outT = nc.declare_dram_parameter("outT", [NH, KS, S], F32, isOutput=True)
    z = nc.declare_dram_parameter("z", [NH, S], F32, isOutput=True)

    with tile.TileContext(nc) as tc, ExitStack() as ctx:
        const_pool = ctx.enter_context(tc.tile_pool(name="const", bufs=1))
        qkv_pool = ctx.enter_context(tc.tile_pool(name="qkv", bufs=1))
        out_pool = ctx.enter_context(tc.tile_pool(name="outp", bufs=1))
        xw_pool = ctx.enter_context(tc.tile_pool(name="xw", bufs=1))
        pexp_pool = ctx.enter_context(tc.tile_pool(name="pexp", bufs=4))
        osb_pool = ctx.enter_context(tc.tile_pool(name="osb", bufs=6))
        wkern_sb = const_pool.tile([KS, NH, KS], mm_dt)
        nc.sync.dma_start(wkern_sb[:], wkern[:].rearrange("h k j -> k h j"))
        mask_sb = const_pool.tile([P, P], mm_dt)
        nc.sync.dma_start(mask_sb[:], masks[:])

        qt_sb = [
            qkv_pool.tile([P, S], mm_dt, tag=f"qt{i}", name=f"qt{i}") for i in range(2)
        ]
        kt_sb = [
            qkv_pool.tile([P, S], mm_dt, tag=f"kt{i}", name=f"kt{i}") for i in range(2)
        ]
        v_sb = qkv_pool.tile([P, ST, NH, KS + 1], mm_dt, tag="v")
        nc.vector.memset(v_sb[:, :, :, KS], 1.0)
        outT_sb = out_pool.tile([KS, NH, S], F32)

        w_sb = {}
        for name, wh in (("q", wq), ("k", wk), ("v", wv)):
            w_sb[name] = xw_pool.tile(
                [P, DT, NW], mm_dt, tag=f"w{name}", name=f"w{name}"
            )
            nc.sync.dma_start(
                w_sb[name][:], wh[:].rearrange("(t p) n -> p t n", p=P)
            )
        xT_sb = xw_pool.tile([P, DT, S], mm_dt, tag="xT")
        for t in range(DT):
            nc.sync.dma_start(xT_sb[:, t, :], xT[t * P : (t + 1) * P, :])

        def proj_qk(pt, pool):
            # Q^T / K^T for head-pair pt: [n, s] layout, pair-stacked
            for ic in range(NIB):
                for wname, dst in (("q", qt_sb), ("k", kt_sb)):
                    ps = pool.tile([P, IB], F32, tag="of", name="ps")
                    for t in range(DT):
                        nc.tensor.matmul(
                            ps[:],
                            w_sb[wname][:, t, pt * P : (pt + 1) * P],
                            xT_sb[:, t, ic * IB : (ic + 1) * IB],
                            start=(t == 0),
                            stop=(t == DT - 1),
                        )
                    nc.vector.tensor_copy(
                        dst[pt][:, ic * IB : (ic + 1) * IB], ps[:]
                    )

        def proj_v(pool):
            # V: natural [s, n] layout, all heads, with ones column
            for st in range(ST):
                ps = pool.tile([P, NW], F32, tag="of", name="ps")
                for t in range(DT):
                    nc.tensor.matmul(
                        ps[:],
                        xT_sb[:, t, st * P : (st + 1) * P],
                        w_sb["v"][:, t, :],
                        start=(t == 0),
                        stop=(t == DT - 1),
                    )
                nc.vector.tensor_copy(
                    v_sb[:, st, :, 0:KS],
                    ps[:].rearrange("p (h k) -> p h k", k=KS),
                )

        def attention(pr, after_ib=None):
            # causal attention + output projection for head pair pr
            # (core heads 2*pr and 2*pr+1), scores row-packed via
            # tile_position so both heads' K=64 matmuls share the PE array
            for ib in range(NIB):
                if after_ib is not None and ib in after_ib:
                    after_ib[ib]()
                nl = (ib + 1) * LPB
                o_ps = [
                    po.tile([KS + 1, IB], F32, tag="of", name=f"o{pr}_{ib}_{hh}")
                    for hh in range(2)
                ]
                for lt in range(nl):
                    # causal: columns [0, off) of this i-block are fully
                    # masked for key tile lt; compute only the suffix
                    off = max(0, (lt - ib * LPB)) * P
                    st_ps = pst.tile([P, 2, IB], F32, tag="st", name="st")
                    for hh in range(2):
                        nc.tensor.matmul(
                            st_ps[:, hh, off:IB],
                            kt_sb[pr][hh * KS : (hh + 1) * KS, lt * P : (lt + 1) * P],
                            qt_sb[pr][
                                hh * KS : (hh + 1) * KS,
                                ib * IB + off : (ib + 1) * IB,
                            ],
                            start=True,
                            stop=True,
                            tile_position=(hh * KS, 0),
                        )
                    pe = pexp_pool.tile([P, 2, IB], BF16, tag="pe", name="pe")
                    nc.scalar.activation(
                        pe[:, :, off:IB], st_ps[:, :, off:IB], EXP, scale=0.125
                    )
                    if lt >= ib * LPB:  # diagonal 128-block -> triangular mask
                        for hh in range(2):
                            nc.vector.tensor_mul(
                                pe[:, hh, off : off + P],
                                pe[:, hh, off : off + P],
                                mask_sb[:],
                            )
                    for hh in range(2):
                        nc.tensor.matmul(
                            o_ps[hh][:, off:IB],
                            v_sb[:, lt, 2 * pr + hh, :],
                            pe[:, hh, off:IB],
                            start=(lt == 0),
                            stop=(lt == nl - 1),
                        )
                for hh in range(2):
                    h = 2 * pr + hh
                    # bf16 rows for the projection matmul, f32 Z row for
                    # exact normalization on the host
                    o_bf = osb_pool.tile([KS, IB], BF16, tag="o_bf", name="o_bf")
                    nc.vector.tensor_copy(o_bf[:], o_ps[hh][0:KS, :])
                    z_sb = osb_pool.tile([KS + 1, IB], F32, tag="z_sb", name="z_sb")
                    nc.vector.tensor_copy(
                        z_sb[KS : KS + 1, :], o_ps[hh][KS : KS + 1, :]
                    )
                    nc.sync.dma_start(
                        z[h, ib * IB : (ib + 1) * IB], z_sb[KS : KS + 1, :]
                    )
                    f_ps = po.tile([KS, IB], F32, tag="of", name="f_ps")
                    nc.tensor.matmul(
                        f_ps[:], wkern_sb[:, h, :], o_bf[:],
                        start=True, stop=True,
                    )
                    nc.vector.tensor_copy(
                        outT_sb[:, h, ib * IB : (ib + 1) * IB], f_ps[:]
                    )
            nc.sync.dma_start(
                outT[:].rearrange("h k s -> k h s")[:, 2 * pr : 2 * pr + 2, :],
                outT_sb[:, 2 * pr : 2 * pr + 2, :],
            )

        # PE warmup: dependency-free matmuls on zeroed scratch so the HAM
        # clock gate reaches 8/8 during the input-DMA lead-in, before real
        # matmuls (which otherwise run the whole projection phase at 1.2 GHz)
        warm_in = const_pool.tile([P, IB], BF16)
        nc.vector.memset(warm_in[:], 0.0)

        with tc.tile_pool(name="pproj", bufs=2, space=bass.MemorySpace.PSUM) as pp:
            for _ in range(45):
                w_ps = pp.tile([P, IB], F32, tag="of", name="w_ps")
                nc.tensor.matmul(
                    w_ps[:], warm_in[:, 0:P], warm_in[:], start=True, stop=True
                )
            proj_qk(0, pp)
            proj_v(pp)
        pst = ctx.enter_context(
            tc.tile_pool(name="pst", bufs=2, space=bass.MemorySpace.PSUM)
        )
        po = ctx.enter_context(
            tc.tile_pool(name="po", bufs=4, space=bass.MemorySpace.PSUM)
        )
        attention(0)
        proj_qk(1, po)  # emitted after attention(0): fills PE gaps during it
        attention(1)

    nc.compile()
    return nc


def make_masks():
    # triangular [P, P]: within a diagonal 128-block keep j >= p
    j = np.arange(P)[None, :]
    p = np.arange(P)[:, None]
    return (j >= p).astype(NP_BF16)


def make_in_maps(inputs):
    x = np.asarray(inputs["x"], np.float32)
    Wq = np.asarray(inputs["Wq"], np.float32)
    Wk = np.asarray(inputs["Wk"], np.float32)
    Wv = np.asarray(inputs["Wv"], np.float32)
    kern = np.asarray(inputs["kernel"], np.float32)

    masks = make_masks()
    kern3 = kern.reshape(KS, H, KS)  # [k, h, j]
    in_maps = []
    for c in range(NCORES):
        b, hs = c // CORES_PER_B, (c % CORES_PER_B) * NH
        in_maps.append(
            {
                "xT": x[b].T.astype(NP_BF16),
                "wq": Wq[:, :, hs : hs + NH].transpose(0, 2, 1).reshape(D, NW)
                .astype(NP_BF16),
                "wk": Wk[:, :, hs : hs + NH].transpose(0, 2, 1).reshape(D, NW)
                .astype(NP_BF16),
                "wv": Wv[:, :, hs : hs + NH].transpose(0, 2, 1).reshape(D, NW)
                .astype(NP_BF16),
                "wkern": kern3[:, hs : hs + NH, :].transpose(1, 0, 2)
                .astype(NP_BF16),
                "masks": masks,
            }
        )
    return in_maps


def gather_output(results):
    out = np.zeros((B, S, KS), np.float32)
    for c in range(NCORES):
        b = c // CORES_PER_B
        oT = np.asarray(results[c]["outT"], np.float32)  # [NH, KS, S]
        zz = np.asarray(results[c]["z"], np.float32)     # [NH, S]
        out[b] += (oT / zz[:, None, :]).sum(axis=0).T
    return out


_NC_CACHE = {}


def get_nc():
    if "nc" not in _NC_CACHE:
        _NC_CACHE["nc"] = build_nc()
    return _NC_CACHE["nc"]


def run_hw(inputs, trace=False, **kw):
    from concourse.bass_utils import run_bass_kernel_spmd

    nc = get_nc()
    in_maps = make_in_maps(inputs)
    res = run_bass_kernel_spmd(
        nc, in_maps, list(range(NCORES)), trace=trace, **kw
    )
    return gather_output(res.results), res


def kernel(**inputs) -> np.ndarray:
    out, _ = run_hw(inputs, trace=False)
    return out



# revision 5
# speedup vs baseline: 1.2115x; 1.2115x over previous
"""Trainium2 Bass kernel for nn_MultiHeadAttention_56375740727430.

Causal multi-head attention, B=2 S=2048 D=1024 H=16 KS=64, followed by an
output projection `heads @ kernel`.

Sharding: pure data/head parallel over 8 cores — core c handles batch c//4
and 4 heads (c%4)*4 ... +4.  Each core computes Q^T/K^T (head-pair-stacked,
transposed layout), V (natural layout, with an appended ones-column so the
softmax denominator Z falls out of the attention matmul), causal scores ->
exp -> (P@V | Z) -> per-head output projection, all unnormalized.  The host
divides by Z, sums head contributions and batch-partials, and transposes.

Matmul operands are bf16 (1 col/cycle on the PE); accumulation, scores (exp
input) and Z stay fp32.

v2 schedule:
- projections emitted t-major (one PSUM accumulator per output chunk, all 8
  banks) so the PE starts as soon as the first x d-tile lands and overlaps
  the input DMA stream;
- attention inner loop software-pipelined: scores(lt+1) is emitted before
  PV(lt), so the PE never sits behind the exp on the critical path and PV
  weight loads prefetch during the preceding score pair;
- Q^T/K^T for the second head pair are produced by filler chunks interleaved
  into attention(0) (2 per query block, matching the PSUM ring);
- the two heads of the output projection run as a block-diagonal pair via
  tile_position (0,0)/(64,64);
- outputs are DMA'd per query block; staging copies run on the otherwise
  idle GpSimd engine.
"""

import sys

sys.path.insert(0, "/opt/trn_rl_repo")

from contextlib import ExitStack

import ml_dtypes
import numpy as np

import concourse.bass as bass
import concourse.bacc as bacc
import concourse.mybir as mybir
import concourse.tile as tile

B, S, D = 2, 2048, 1024
H, KS = 16, 64

P = 128            # partitions
NCORES = 8
CORES_PER_B = NCORES // B          # 4
NH = H // CORES_PER_B              # heads per core = 4
NW = NH * KS                       # per-core projection width = 256
DT = D // P                        # d-tiles = 8
ST = S // P                        # s/l-tiles = 16
IB = 512                           # query block
NIB = S // IB                      # 4
LPB = IB // P                      # l-tiles per query block = 4
NWARM = 6                          # PE clock-ramp matmuls

F32 = mybir.dt.float32
BF16 = mybir.dt.bfloat16
NP_BF16 = ml_dtypes.bfloat16
EXP = mybir.ActivationFunctionType.Exp


def build_nc():
    mm_dt = BF16
    nc = bacc.Bacc()

    xT = nc.declare_dram_parameter("xT", [D, S], mm_dt, isOutput=False)
    wq = nc.declare_dram_parameter("wq", [D, NW], mm_dt, isOutput=False)
    wk = nc.declare_dram_parameter("wk", [D, NW], mm_dt, isOutput=False)
    wv = nc.declare_dram_parameter("wv", [D, NW], mm_dt, isOutput=False)
    # pair layout: partition = hh*KS + k, dims = [head pair, j]
    wkern = nc.declare_dram_parameter("wkern", [P, NH // 2, KS], mm_dt, isOutput=False)
    masks = nc.declare_dram_parameter("masks", [P, P], mm_dt, isOutput=False)
    outT = nc.declare_dram_parameter("outT", [NH, KS, S], mm_dt, isOutput=True)
    z = nc.declare_dram_parameter("z", [NH, S], F32, isOutput=True)

    with tile.TileContext(nc) as tc, ExitStack() as ctx:
        const_pool = ctx.enter_context(tc.tile_pool(name="const", bufs=1))
        qkv_pool = ctx.enter_context(tc.tile_pool(name="qkv", bufs=1))
        out_pool = ctx.enter_context(tc.tile_pool(name="outp", bufs=1))
        xw_pool = ctx.enter_context(tc.tile_pool(name="xw", bufs=1))
        pexp_pool = ctx.enter_context(tc.tile_pool(name="pexp", bufs=4))
        osb_pool = ctx.enter_context(tc.tile_pool(name="osb", bufs=4))

        warm_in = const_pool.tile([P, IB], mm_dt)
        nc.gpsimd.memset(warm_in[:], 0.0)

        qt_sb = [
            qkv_pool.tile([P, S], mm_dt, tag=f"qt{i}", name=f"qt{i}") for i in range(2)
        ]
        kt_sb = [
            qkv_pool.tile([P, S], mm_dt, tag=f"kt{i}", name=f"kt{i}") for i in range(2)
        ]
        v_sb = qkv_pool.tile([P, ST, NH, KS + 1], mm_dt, tag="v")
        nc.gpsimd.memset(v_sb[:, :, :, KS], 1.0)
        outT_sb = out_pool.tile([KS, NH, S], mm_dt)

        # input DMA: small consts first, then per-d-tile weight + x slices in
        # t order so the t-major projection waves start as soon as possible
        wkern_sb = const_pool.tile([P, NH // 2, KS], mm_dt)
        nc.sync.dma_start(wkern_sb[:], wkern[:])
        mask_sb = const_pool.tile([P, P], mm_dt)
        nc.sync.dma_start(mask_sb[:], masks[:])

        w_sb = {}
        for name, wh in (("q", wq), ("k", wk), ("v", wv)):
            w_sb[name] = xw_pool.tile(
                [P, DT, NW], mm_dt, tag=f"w{name}", name=f"w{name}"
            )
        xT_sb = xw_pool.tile([P, DT, S], mm_dt, tag="xT")
        for t in range(DT):
            for name, wh in (("q", wq), ("k", wk), ("v", wv)):
                nc.sync.dma_start(
                    w_sb[name][:, t, :], wh[t * P : (t + 1) * P, :]
                )
            nc.sync.dma_start(xT_sb[:, t, :], xT[t * P : (t + 1) * P, :])

        # ---- projection phase: t-major waves over 8 PSUM accumulators ----
        with tc.tile_pool(name="pqk", bufs=8, space=bass.MemorySpace.PSUM) as pqk:
            for _ in range(NWARM):
                w_ps = pqk.tile([P, IB], F32, tag="of", name="w_ps")
                nc.tensor.matmul(
                    w_ps[:], warm_in[:, 0:P], warm_in[:], start=True, stop=True
                )
            chunks = [(wn, ic) for wn in ("q", "k") for ic in range(NIB)]
            ps = {
                c: pqk.tile([P, IB], F32, tag="of", name=f"qk0_{c[0]}{c[1]}")
                for c in chunks
            }
            for t in range(DT):
                for wn, ic in chunks:
                    nc.tensor.matmul(
                        ps[(wn, ic)][:],
                        w_sb[wn][:, t, 0:P],
                        xT_sb[:, t, ic * IB : (ic + 1) * IB],
                        start=(t == 0),
                        stop=(t == DT - 1),
                    )
                    if t == DT - 1:
                        dst = qt_sb if wn == "q" else kt_sb
                        nc.vector.tensor_copy(
                            dst[0][:, ic * IB : (ic + 1) * IB], ps[(wn, ic)][:]
                        )
        with tc.tile_pool(name="pv", bufs=8, space=bass.MemorySpace.PSUM) as pvp:
            for g in range(2):
                sts = range(g * 8, (g + 1) * 8)
                vs = {
                    st: pvp.tile([P, NW], F32, tag="v", name=f"v{st}") for st in sts
                }
                for t in range(DT):
                    for st in sts:
                        nc.tensor.matmul(
                            vs[st][:],
                            xT_sb[:, t, st * P : (st + 1) * P],
                            w_sb["v"][:, t, :],
                            start=(t == 0),
                            stop=(t == DT - 1),
                        )
                        if t == DT - 1:
                            nc.vector.tensor_copy(
                                v_sb[:, st, :, 0:KS],
                                vs[st][:].rearrange("p (h k) -> p h k", k=KS),
                            )

        pst = ctx.enter_context(
            tc.tile_pool(name="pst", bufs=2, space=bass.MemorySpace.PSUM)
        )
        po = ctx.enter_context(
            tc.tile_pool(name="po", bufs=4, space=bass.MemorySpace.PSUM)
        )

        def qk1_chunk(wn, ic):
            cps = po.tile([P, IB], F32, tag="of", name=f"qk1_{wn}{ic}")
            for t in range(DT):
                nc.tensor.matmul(
                    cps[:],
                    w_sb[wn][:, t, P : 2 * P],
                    xT_sb[:, t, ic * IB : (ic + 1) * IB],
                    start=(t == 0),
                    stop=(t == DT - 1),
                )
            dst = qt_sb if wn == "q" else kt_sb
            nc.vector.tensor_copy(dst[1][:, ic * IB : (ic + 1) * IB], cps[:])

        def attention(pr, fillers):
            # causal attention + output projection for head pair pr
            # (core heads 2*pr and 2*pr+1); scores row-packed via
            # tile_position so both heads' K=64 matmuls share the PE array.
            # Inner loop is software-pipelined: PV(lt) is emitted after
            # scores(lt+1) so the PE isn't gated on exp(lt).
            for ib in range(NIB):
                nl = (ib + 1) * LPB
                o_ps = [
                    po.tile([KS + 1, IB], F32, tag="of", name=f"o{pr}_{ib}_{hh}")
                    for hh in range(2)
                ]

                def emit_pv(lt, pe, off):
                    for hh in range(2):
                        nc.tensor.matmul(
                            o_ps[hh][:, off:IB],
                            v_sb[:, lt, 2 * pr + hh, :],
                            pe[:, hh, off:IB],
                            start=(lt == 0),
                            stop=(lt == nl - 1),
                        )

                pending = None
                for lt in range(nl):
                    # causal: columns [0, off) of this i-block are fully
                    # masked for key tile lt; compute only the suffix
                    off = max(0, (lt - ib * LPB)) * P
                    st_ps = pst.tile([P, 2, IB], F32, tag="st", name="st")
                    for hh in range(2):
                        nc.tensor.matmul(
                            st_ps[:, hh, off:IB],
                            kt_sb[pr][hh * KS : (hh + 1) * KS, lt * P : (lt + 1) * P],
                            qt_sb[pr][
                                hh * KS : (hh + 1) * KS,
                                ib * IB + off : (ib + 1) * IB,
                            ],
                            start=True,
                            stop=True,
                            tile_position=(hh * KS, 0),
                        )
                    pe = pexp_pool.tile([P, 2, IB], BF16, tag="pe", name="pe")
                    nc.scalar.activation(
                        pe[:, :, off:IB], st_ps[:, :, off:IB], EXP, scale=0.125
                    )
                    if lt >= ib * LPB:  # diagonal 128-block -> triangular mask
                        for hh in range(2):
                            nc.vector.tensor_mul(
                                pe[:, hh, off : off + P],
                                pe[:, hh, off : off + P],
                                mask_sb[:],
                            )
                    if pending is not None:
                        emit_pv(*pending)
                    if lt in (1, 3) and fillers:
                        fillers.pop(0)()
                    pending = (lt, pe, off)
                emit_pv(*pending)

                # both heads' output projections as one block-diagonal pair
                o_bf = osb_pool.tile([P, IB], BF16, tag="o_bf", name="o_bf")
                for hh in range(2):
                    z_sb = osb_pool.tile([1, IB], F32, tag="z_sb", name="z_sb")
                    nc.vector.tensor_copy(
                        o_bf[hh * KS : (hh + 1) * KS, :], o_ps[hh][0:KS, :]
                    )
                    nc.vector.tensor_copy(
                        z_sb[0:1, :], o_ps[hh][KS : KS + 1, :]
                    )
                    nc.sync.dma_start(
                        z[2 * pr + hh, ib * IB : (ib + 1) * IB], z_sb[0:1, :]
                    )
                f_ps = po.tile([P, IB], F32, tag="of", name="f_ps")
                for hh in range(2):
                    nc.tensor.matmul(
                        f_ps[hh * KS : (hh + 1) * KS, :],
                        wkern_sb[hh * KS : (hh + 1) * KS, pr, :],
                        o_bf[hh * KS : (hh + 1) * KS, :],
                        start=True,
                        stop=True,
                        tile_position=(hh * KS, hh * KS),
                    )
                for hh in range(2):
                    nc.vector.tensor_copy(
                        outT_sb[:, 2 * pr + hh, ib * IB : (ib + 1) * IB],
                        f_ps[hh * KS : (hh + 1) * KS, :],
                    )
                nc.sync.dma_start(
                    outT[:]
                    .rearrange("h k s -> k h s")[
                        :, 2 * pr : 2 * pr + 2, ib * IB : (ib + 1) * IB
                    ],
                    outT_sb[:, 2 * pr : 2 * pr + 2, ib * IB : (ib + 1) * IB],
                )

        fillers = [
            (lambda wn=wn, ic=ic: qk1_chunk(wn, ic))
            for ic in range(NIB)
            for wn in ("q", "k")
        ]
        attention(0, fillers)
        while fillers:  # any chunks that didn't fit the interleave slots
            fillers.pop(0)()
        attention(1, [])

    nc.compile()
    return nc


def make_masks():
    # triangular [P, P]: within a diagonal 128-block keep j >= p
    j = np.arange(P)[None, :]
    p = np.arange(P)[:, None]
    return (j >= p).astype(NP_BF16)


def make_in_maps(inputs):
    x = np.asarray(inputs["x"], np.float32)
    Wq = np.asarray(inputs["Wq"], np.float32)
    Wk = np.asarray(inputs["Wk"], np.float32)
    Wv = np.asarray(inputs["Wv"], np.float32)
    kern = np.asarray(inputs["kernel"], np.float32)

    masks = make_masks()
    kern3 = kern.reshape(KS, H, KS)  # [k, h, j]
    in_maps = []
    for c in range(NCORES):
        b, hs = c // CORES_PER_B, (c % CORES_PER_B) * NH
        # wkern pair layout: [hh*KS + k, pair, j] for heads h = hs + 2*pair + hh
        kern_c = kern3[:, hs : hs + NH, :]  # [k, h, j]
        wkern_pair = np.zeros((P, NH // 2, KS), np.float32)
        for pair in range(NH // 2):
            for hh in range(2):
                wkern_pair[hh * KS : (hh + 1) * KS, pair, :] = kern_c[
                    :, 2 * pair + hh, :
                ]
        in_maps.append(
            {
                "xT": x[b].T.astype(NP_BF16),
                "wq": Wq[:, :, hs : hs + NH].transpose(0, 2, 1).reshape(D, NW)
                .astype(NP_BF16),
                "wk": Wk[:, :, hs : hs + NH].transpose(0, 2, 1).reshape(D, NW)
                .astype(NP_BF16),
                "wv": Wv[:, :, hs : hs + NH].transpose(0, 2, 1).reshape(D, NW)
                .astype(NP_BF16),
                "wkern": wkern_pair.astype(NP_BF16),
                "masks": masks,
            }
        )
    return in_maps


def gather_output(results):
    out = np.zeros((B, S, KS), np.float32)
    for c in range(NCORES):
        b = c // CORES_PER_B
        oT = np.asarray(results[c]["outT"], np.float32)  # [NH, KS, S]
        zz = np.asarray(results[c]["z"], np.float32)     # [NH, S]
        out[b] += (oT / zz[:, None, :]).sum(axis=0).T
    return out


_NC_CACHE = {}


def get_nc():
    if "nc" not in _NC_CACHE:
        _NC_CACHE["nc"] = build_nc()
    return _NC_CACHE["nc"]


def run_hw(inputs, trace=False, **kw):
    from concourse.bass_utils import run_bass_kernel_spmd

    nc = get_nc()
    in_maps = make_in_maps(inputs)
    res = run_bass_kernel_spmd(
        nc, in_maps, list(range(NCORES)), trace=trace, **kw
    )
    return gather_output(res.results), res


def kernel(**inputs) -> np.ndarray:
    out, _ = run_hw(inputs, trace=False)
    return out


# revision 7
# speedup vs baseline: 1.2180x; 1.0054x over previous
"""Trainium2 Bass kernel for nn_MultiHeadAttention_56375740727430.

Causal multi-head attention, B=2 S=2048 D=1024 H=16 KS=64, followed by an
output projection `heads @ kernel`.

Sharding: pure data/head parallel over 8 cores — core c handles batch c//4
and 4 heads (c%4)*4 ... +4.  Each core computes Q^T/K^T (head-pair-stacked,
transposed layout), V (natural layout, with an appended ones-column so the
softmax denominator Z falls out of the attention matmul), causal scores ->
exp -> (P@V | Z) -> per-head output projection, all unnormalized.  The host
divides by Z, sums head contributions and batch-partials, and transposes.

Matmul operands are bf16 (1 col/cycle on the PE); accumulation, scores (exp
input) and Z stay fp32.

v2 schedule:
- projections emitted t-major (one PSUM accumulator per output chunk, all 8
  banks) so the PE starts as soon as the first x d-tile lands and overlaps
  the input DMA stream;
- attention inner loop software-pipelined: scores(lt+1) is emitted before
  PV(lt), so the PE never sits behind the exp on the critical path and PV
  weight loads prefetch during the preceding score pair;
- Q^T/K^T for the second head pair are produced by filler chunks interleaved
  into attention(0) (2 per query block, matching the PSUM ring);
- the two heads of the output projection run as a block-diagonal pair via
  tile_position (0,0)/(64,64);
- outputs are DMA'd per query block; staging copies run on the otherwise
  idle GpSimd engine.
"""

import sys

sys.path.insert(0, "/opt/trn_rl_repo")

from contextlib import ExitStack

import ml_dtypes
import numpy as np

import concourse.bass as bass
import concourse.bacc as bacc
import concourse.mybir as mybir
import concourse.tile as tile

B, S, D = 2, 2048, 1024
H, KS = 16, 64

P = 128            # partitions
NCORES = 8
CORES_PER_B = NCORES // B          # 4
NH = H // CORES_PER_B              # heads per core = 4
NW = NH * KS                       # per-core projection width = 256
DT = D // P                        # d-tiles = 8
ST = S // P                        # s/l-tiles = 16
IB = 512                           # query block
NIB = S // IB                      # 4
LPB = IB // P                      # l-tiles per query block = 4
NWARM = 6                          # PE clock-ramp matmuls

F32 = mybir.dt.float32
BF16 = mybir.dt.bfloat16
NP_BF16 = ml_dtypes.bfloat16
EXP = mybir.ActivationFunctionType.Exp


def build_nc():
    mm_dt = BF16
    nc = bacc.Bacc()

    xT = nc.declare_dram_parameter("xT", [D, S], mm_dt, isOutput=False)
    wq = nc.declare_dram_parameter("wq", [D, NW], mm_dt, isOutput=False)
    wk = nc.declare_dram_parameter("wk", [D, NW], mm_dt, isOutput=False)
    wv = nc.declare_dram_parameter("wv", [D, NW], mm_dt, isOutput=False)
    # pair layout: partition = hh*KS + k, dims = [head pair, j]
    wkern = nc.declare_dram_parameter("wkern", [P, NH // 2, KS], mm_dt, isOutput=False)
    masks = nc.declare_dram_parameter("masks", [P, P], mm_dt, isOutput=False)
    outT = nc.declare_dram_parameter("outT", [NH, KS, S], mm_dt, isOutput=True)
    z = nc.declare_dram_parameter("z", [NH, S], F32, isOutput=True)

    with tile.TileContext(nc) as tc, ExitStack() as ctx:
        const_pool = ctx.enter_context(tc.tile_pool(name="const", bufs=1))
        qkv_pool = ctx.enter_context(tc.tile_pool(name="qkv", bufs=1))
        out_pool = ctx.enter_context(tc.tile_pool(name="outp", bufs=1))
        xw_pool = ctx.enter_context(tc.tile_pool(name="xw", bufs=1))
        pexp_pool = ctx.enter_context(tc.tile_pool(name="pexp", bufs=4))
        osb_pool = ctx.enter_context(tc.tile_pool(name="osb", bufs=4))

        warm_in = const_pool.tile([P, IB], mm_dt)
        nc.gpsimd.memset(warm_in[:], 0.0)

        qt_sb = [
            qkv_pool.tile([P, S], mm_dt, tag=f"qt{i}", name=f"qt{i}") for i in range(2)
        ]
        kt_sb = [
            qkv_pool.tile([P, S], mm_dt, tag=f"kt{i}", name=f"kt{i}") for i in range(2)
        ]
        v_sb = qkv_pool.tile([P, ST, NH, KS + 1], mm_dt, tag="v")
        nc.gpsimd.memset(v_sb[:, :, :, KS], 1.0)
        outT_sb = out_pool.tile([KS, NH, S], mm_dt)

        # input DMA: small consts first, then per-d-tile weight + x slices in
        # t order so the t-major projection waves start as soon as possible
        wkern_sb = const_pool.tile([P, NH // 2, KS], mm_dt)
        nc.sync.dma_start(wkern_sb[:], wkern[:])
        mask_sb = const_pool.tile([P, P], mm_dt)
        nc.sync.dma_start(mask_sb[:], masks[:])

        w_sb = {}
        for name, wh in (("q", wq), ("k", wk), ("v", wv)):
            w_sb[name] = xw_pool.tile(
                [P, DT, NW], mm_dt, tag=f"w{name}", name=f"w{name}"
            )
        xT_sb = xw_pool.tile([P, DT, S], mm_dt, tag="xT")
        for t in range(DT):
            for name, wh in (("q", wq), ("k", wk), ("v", wv)):
                nc.sync.dma_start(
                    w_sb[name][:, t, :], wh[t * P : (t + 1) * P, :]
                )
            nc.sync.dma_start(xT_sb[:, t, :], xT[t * P : (t + 1) * P, :])

        # ---- projection phase: t-major waves over 8 PSUM accumulators.
        # One pool for warmup + Q/K waves + V waves so V's accumulators
        # pipeline into slots as the Q/K copies retire (no barrier).
        with tc.tile_pool(name="pproj", bufs=8, space=bass.MemorySpace.PSUM) as pproj:
            for _ in range(NWARM):
                w_ps = pproj.tile([P, IB], F32, tag="of", name="w_ps")
                nc.tensor.matmul(
                    w_ps[:], warm_in[:, 0:P], warm_in[:], start=True, stop=True
                )
            chunks = [(wn, ic) for wn in ("q", "k") for ic in range(NIB)]
            ps = {
                c: pproj.tile([P, IB], F32, tag="of", name=f"qk0_{c[0]}{c[1]}")
                for c in chunks
            }
            for t in range(DT):
                for wn, ic in chunks:
                    nc.tensor.matmul(
                        ps[(wn, ic)][:],
                        w_sb[wn][:, t, 0:P],
                        xT_sb[:, t, ic * IB : (ic + 1) * IB],
                        start=(t == 0),
                        stop=(t == DT - 1),
                    )
                    if t == DT - 1:
                        dst = qt_sb if wn == "q" else kt_sb
                        nc.vector.tensor_copy(
                            dst[0][:, ic * IB : (ic + 1) * IB], ps[(wn, ic)][:]
                        )
            for g in range(2):
                sts = range(g * 8, (g + 1) * 8)
                vs = {
                    st: pproj.tile([P, NW], F32, tag="of", name=f"v{st}")
                    for st in sts
                }
                for t in range(DT):
                    for st in sts:
                        nc.tensor.matmul(
                            vs[st][:],
                            xT_sb[:, t, st * P : (st + 1) * P],
                            w_sb["v"][:, t, :],
                            start=(t == 0),
                            stop=(t == DT - 1),
                        )
                        if t == DT - 1:
                            nc.vector.tensor_copy(
                                v_sb[:, st, :, 0:KS],
                                vs[st][:].rearrange("p (h k) -> p h k", k=KS),
                            )

        pst = ctx.enter_context(
            tc.tile_pool(name="pst", bufs=2, space=bass.MemorySpace.PSUM)
        )
        po = ctx.enter_context(
            tc.tile_pool(name="po", bufs=4, space=bass.MemorySpace.PSUM)
        )

        def qk1_chunk(wn, ic):
            cps = po.tile([P, IB], F32, tag="of", name=f"qk1_{wn}{ic}")
            for t in range(DT):
                nc.tensor.matmul(
                    cps[:],
                    w_sb[wn][:, t, P : 2 * P],
                    xT_sb[:, t, ic * IB : (ic + 1) * IB],
                    start=(t == 0),
                    stop=(t == DT - 1),
                )
            dst = qt_sb if wn == "q" else kt_sb
            nc.vector.tensor_copy(dst[1][:, ic * IB : (ic + 1) * IB], cps[:])

        def attention(pr, fillers):
            # causal attention + output projection for head pair pr
            # (core heads 2*pr and 2*pr+1); scores row-packed via
            # tile_position so both heads' K=64 matmuls share the PE array.
            # Inner loop is software-pipelined: PV(lt) is emitted after
            # scores(lt+1) so the PE isn't gated on exp(lt).
            for ib in range(NIB):
                nl = (ib + 1) * LPB
                o_ps = [
                    po.tile([KS + 1, IB], F32, tag="of", name=f"o{pr}_{ib}_{hh}")
                    for hh in range(2)
                ]

                def emit_pv(lt, pe, off):
                    for hh in range(2):
                        nc.tensor.matmul(
                            o_ps[hh][:, off:IB],
                            v_sb[:, lt, 2 * pr + hh, :],
                            pe[:, hh, off:IB],
                            start=(lt == 0),
                            stop=(lt == nl - 1),
                        )

                pending = None
                for lt in range(nl):
                    # causal: columns [0, off) of this i-block are fully
                    # masked for key tile lt; compute only the suffix
                    off = max(0, (lt - ib * LPB)) * P
                    st_ps = pst.tile([P, 2, IB], F32, tag="st", name="st")
                    for hh in range(2):
                        nc.tensor.matmul(
                            st_ps[:, hh, off:IB],
                            kt_sb[pr][hh * KS : (hh + 1) * KS, lt * P : (lt + 1) * P],
                            qt_sb[pr][
                                hh * KS : (hh + 1) * KS,
                                ib * IB + off : (ib + 1) * IB,
                            ],
                            start=True,
                            stop=True,
                            tile_position=(hh * KS, 0),
                        )
                    pe = pexp_pool.tile([P, 2, IB], BF16, tag="pe", name="pe")
                    nc.scalar.activation(
                        pe[:, :, off:IB], st_ps[:, :, off:IB], EXP, scale=0.125
                    )
                    if lt >= ib * LPB:  # diagonal 128-block -> triangular mask
                        for hh in range(2):
                            nc.vector.tensor_mul(
                                pe[:, hh, off : off + P],
                                pe[:, hh, off : off + P],
                                mask_sb[:],
                            )
                    if pending is not None:
                        emit_pv(*pending)
                    if lt in (1, 3) and fillers:
                        fillers.pop(0)()
                    pending = (lt, pe, off)
                emit_pv(*pending)

                # both heads' output projections as one block-diagonal pair
                o_bf = osb_pool.tile([P, IB], BF16, tag="o_bf", name="o_bf")
                for hh in range(2):
                    z_sb = osb_pool.tile([1, IB], F32, tag="z_sb", name="z_sb")
                    nc.vector.tensor_copy(
                        o_bf[hh * KS : (hh + 1) * KS, :], o_ps[hh][0:KS, :]
                    )
                    nc.vector.tensor_copy(
                        z_sb[0:1, :], o_ps[hh][KS : KS + 1, :]
                    )
                    nc.sync.dma_start(
                        z[2 * pr + hh, ib * IB : (ib + 1) * IB], z_sb[0:1, :]
                    )
                f_ps = po.tile([P, IB], F32, tag="of", name="f_ps")
                for hh in range(2):
                    nc.tensor.matmul(
                        f_ps[hh * KS : (hh + 1) * KS, :],
                        wkern_sb[hh * KS : (hh + 1) * KS, pr, :],
                        o_bf[hh * KS : (hh + 1) * KS, :],
                        start=True,
                        stop=True,
                        tile_position=(hh * KS, hh * KS),
                    )
                for hh in range(2):
                    h = 2 * pr + hh
                    nc.vector.tensor_copy(
                        outT_sb[:, h, ib * IB : (ib + 1) * IB],
                        f_ps[hh * KS : (hh + 1) * KS, :],
                    )
                    nc.sync.dma_start(
                        outT[h, :, ib * IB : (ib + 1) * IB],
                        outT_sb[:, h, ib * IB : (ib + 1) * IB],
                    )

        fillers = [
            (lambda wn=wn, ic=ic: qk1_chunk(wn, ic))
            for ic in range(NIB)
            for wn in ("q", "k")
        ]
        attention(0, fillers)
        while fillers:  # any chunks that didn't fit the interleave slots
            fillers.pop(0)()
        attention(1, [])

    nc.compile()
    return nc


def make_masks():
    # triangular [P, P]: within a diagonal 128-block keep j >= p
    j = np.arange(P)[None, :]
    p = np.arange(P)[:, None]
    return (j >= p).astype(NP_BF16)


def make_in_maps(inputs):
    x = np.asarray(inputs["x"], np.float32)
    Wq = np.asarray(inputs["Wq"], np.float32)
    Wk = np.asarray(inputs["Wk"], np.float32)
    Wv = np.asarray(inputs["Wv"], np.float32)
    kern = np.asarray(inputs["kernel"], np.float32)

    masks = make_masks()
    kern3 = kern.reshape(KS, H, KS)  # [k, h, j]
    in_maps = []
    for c in range(NCORES):
        b, hs = c // CORES_PER_B, (c % CORES_PER_B) * NH
        # wkern pair layout: [hh*KS + k, pair, j] for heads h = hs + 2*pair + hh
        kern_c = kern3[:, hs : hs + NH, :]  # [k, h, j]
        wkern_pair = np.zeros((P, NH // 2, KS), np.float32)
        for pair in range(NH // 2):
            for hh in range(2):
                wkern_pair[hh * KS : (hh + 1) * KS, pair, :] = kern_c[
                    :, 2 * pair + hh, :
                ]
        in_maps.append(
            {
                "xT": x[b].T.astype(NP_BF16),
                "wq": Wq[:, :, hs : hs + NH].transpose(0, 2, 1).reshape(D, NW)
                .astype(NP_BF16),
                "wk": Wk[:, :, hs : hs + NH].transpose(0, 2, 1).reshape(D, NW)
                .astype(NP_BF16),
                "wv": Wv[:, :, hs : hs + NH].transpose(0, 2, 1).reshape(D, NW)
                .astype(NP_BF16),
                "wkern": wkern_pair.astype(NP_BF16),
                "masks": masks,
            }
        )
    return in_maps


def gather_output(results):
    out = np.zeros((B, S, KS), np.float32)
    for c in range(NCORES):
        b = c // CORES_PER_B
        oT = np.asarray(results[c]["outT"], np.float32)  # [NH, KS, S]
        zz = np.asarray(results[c]["z"], np.float32)     # [NH, S]
        out[b] += (oT / zz[:, None, :]).sum(axis=0).T
    return out


_NC_CACHE = {}


def get_nc():
    if "nc" not in _NC_CACHE:
        _NC_CACHE["nc"] = build_nc()
    return _NC_CACHE["nc"]


def run_hw(inputs, trace=False, **kw):
    from concourse.bass_utils import run_bass_kernel_spmd

    nc = get_nc()
    in_maps = make_in_maps(inputs)
    res = run_bass_kernel_spmd(
        nc, in_maps, list(range(NCORES)), trace=trace, **kw
    )
    return gather_output(res.results), res


def kernel(**inputs) -> np.ndarray:
    out, _ = run_hw(inputs, trace=False)
    return out
